# revision 19
# baseline (speedup 1.0000x reference)
"""Trainium2 Bass kernel for nn_CapsNet_69114613730132.

Strategy (8 NeuronCores, SPMD, zero collectives):
  The CapsNet routing loop is degenerate (self.bij is never updated, so
  cij stays 1/512) and collapses to: conv1 -> conv2 -> squash ->
  4096->160 matvec -> elementwise squash. The convolutions are tiny, so
  cross-core collectives (AllGather floor + a ~40us rank-alignment
  barrier measured on this fabric) cost more than replicating them.

  * Every core computes conv1 + conv2 (PrimaryCaps) + squash redundantly:
      conv1 as a 244-contraction matmul over a host-built im2col of x
      (row 243 = ones X conv_b, folding the bias into the matmul);
      conv2 as 81 (dy,dx) PSUM-accumulated matmuls over strided views of
      h (no im2col materialization), weights stationary, bf16; pri_b
      folded in via two 1-row matmuls against a ones vector.
  * The DigitCaps matvec output (160 = 10*16) is sharded 20-per-core via
    per-core weight slices => cores are fully independent; the host just
    concatenates the 8 (1,20) results. No communication at all.
  * All PE compute in bf16 (weights host-cast), f32 PSUM/vector math.

Perf notes (vs the 36us single-ring baseline):
  * The kernel is bound by the 5.3MB conv2 weight stream from HBM
    (~13us at the ~420GB/s per-core aggregate DMA peak). The 16 DMA
    engines time-slice all active rings, so extra rings don't add
    bandwidth -- but the small-packet c1/v transfers on the same ring
    as w2 stalled the stream and (worse) the scalar ring was so slow
    that c1_b/v landed after 24us. Now: all w2 chunks stream on the
    sync ring; c1/pbw/v go on the gpsimd ring early; the scalar ring
    carries nothing so its act-table loads stay off the critical path.
  * Only Sqrt activations are used (one table set): a second table set
    would be reloaded mid-squash-chain (~1.3us stall on first use).
  * Chunks shrink toward the tail (last four are 1 dydx = 65KB) so the
    PE's in-order accumulation trails the final byte by ~0.2us.

kernel(**inputs) takes the FULL unsharded inputs and returns the full
(1,1,10,16,1) float32 output.
"""
import numpy as np
import ml_dtypes

import concourse.bass as bass
import concourse.bacc as bacc
import concourse.tile as tile
import concourse.mybir as mybir
from concourse.bass_utils import run_bass_kernel_spmd
from concourse.tile import ScopedClock, add_dep_helper

FAST_TAIL = True
DEBUG_TAPS = False   # set True (before build_nc) to export x2b/u/s taps


class FastTailTileContext(tile.TileContext):
    """TileContext tail with a 1-hop handshake instead of the all-engine
    barriers (each an EVSEM polling butterfly measured at ~7us here).

    The sync.drain waits for every tracked semaphore target, so by the
    time it passes, every sem-touching instruction on every engine has
    retired (each engine's last real work is upstream of the output DMA
    the drain waits on). A single drain->GpSimd semaphore hop then orders
    the sem/DMA-state clears; the next execution's NEFF entry barrier
    orders everything else."""

    def _drain_and_barrier(self, tick_clock, wait_clock):
        if not FAST_TAIL:
            return super()._drain_and_barrier(tick_clock, wait_clock)
        nc = self.nc
        # GpSimd (the clearing engine, otherwise idle here) waits on every
        # tracked semaphore's final value itself, then clears.
        drain_inst = nc.gpsimd.drain()
        wait_clock.add_sem_waits(
            drain_inst.ins, ScopedClock({None: tick_clock.global_clock})
        )
        # DMA completion sems update asynchronously and the final DMA (the
        # output store) has no downstream consumer, so the clock misses it:
        # wait each DMA-lane sem out to its summed final value explicitly.
        dma_totals = {}
        for insts in self.ordered_instructions_by_block.values():
            for i in insts:
                si = i.sync_info
                if si is None or not si.on_update:
                    continue
                for u in si.on_update:
                    if (u.sync_type == "semaphore" and u.update_value
                            and (u.ant_name or "").startswith("DMA")):
                        k = (u.id, u.ant_name)
                        dma_totals[k] = dma_totals.get(k, 0) + u.update_value
        # Only the clearing engine (gpsimd) must see the final values, and
        # only for lanes whose last update nothing in the program awaited
        # (i.e. the output store): every other lane's total was already
        # covered by the drain's wait clock.
        dma_waited = {}
        for insts in self.ordered_instructions_by_block.values():
            for i in insts:
                si = i.sync_info
                if si is None or not si.on_wait:
                    continue
                for w in si.on_wait:
                    if w.sync_type == "semaphore" and (w.ant_name or "").startswith("DMA"):
                        k = (w.id, w.ant_name)
                        dma_waited[k] = max(dma_waited.get(k, 0),
                                            w.wait_value or 0)
        handles = {h.num: h for h in self.sems.allocated().values()}
        for (sid, nm), tot in sorted(dma_totals.items()):
            if sid in handles and dma_waited.get((sid, nm), 0) < tot:
                nc.gpsimd.wait_ge(handles[sid], tot)
        popped = nc._tile_sem_poison_stack.pop()
        assert popped is self._sem_poison
        nc.clear_and_free_semaphores(list(self.sems.allocated().values()))

BF16 = ml_dtypes.bfloat16
FP8E3 = ml_dtypes.float8_e3m4
F32 = mybir.dt.float32
BF = mybir.dt.bfloat16
F8E3 = mybir.dt.float8e3

NCORES = 8
KI = 20             # digitcaps output elems per core (160 = 8*20)
# dydx positions per w2 DMA chunk, all on the sync ring. Descending
# sizes: big chunks amortize descriptor issue (~650ns each), the 1-dydx
# tail chunks keep the PE right behind the stream end.
W2CHUNKS = [24, 24, 16, 8, 4, 3, 2]
assert sum(W2CHUNKS) == 81
# conv2 weights stream as fp8 E3M4 (4 mantissa bits), scaled by W2SCALE on
# the host into E3M4's normal range (values ~N(0, 0.02)*128 => ~N(0, 2.56),
# |max| ~ 13 < 15.5) and descaled at the PSUM->SBUF copy. The PE upcasts
# each operand independently, so h stays bf16. Halves the HBM stream.
W2SCALE = 128.0
# dydx index after which the two pri_b bias matmuls are inserted into the
# conv2 PSUM accumulation (late enough that pbw's gpsimd-ring DMA has
# landed, well before the stream ends).
BIAS_AT = 48


# --------------------------------------------------------------------------
# Host-side input marshalling (pure layout transforms + dtype casts)
# --------------------------------------------------------------------------

def _host_prep(x, conv_w, conv_b, pri_w, pri_b, W):
    x = np.asarray(x, np.float32)
    conv_w = np.asarray(conv_w, np.float32)
    conv_b = np.asarray(conv_b, np.float32)
    pri_w = np.asarray(pri_w, np.float32)
    pri_b = np.asarray(pri_b, np.float32)
    W = np.asarray(W, np.float32)

    # im2col of x: (243, 256), row (c,dy,dx), col (oy*16+ox)
    im2col1 = np.empty((3, 9, 9, 16, 16), np.float32)
    for dy in range(9):
        for dx in range(9):
            im2col1[:, dy, dx] = x[0, :, dy:dy + 16, dx:dx + 16]
    im2col1 = im2col1.reshape(243, 256)

    W1T = conv_w.reshape(128, 243).T  # (243, 128)

    # 244th contraction row folds conv_b into the conv1 matmul:
    # im2col row = ones, weight row = conv_b.
    im2col_e = np.concatenate([im2col1, np.ones((1, 256), np.float32)])
    W1T_e = np.concatenate([W1T, conv_b.reshape(1, 128)])

    # (ic, dydx*256 + oc2) with oc2 = cap*8 + j
    w2s = (pri_w.reshape(256, 128, 9, 9)
           .transpose(2, 3, 1, 0)          # (dy, dx, ic, oc2)
           .reshape(81, 128, 256)
           .transpose(1, 0, 2)             # (ic, dydx, oc2)
           .reshape(128, 81 * 256))
    w2s = np.clip(w2s * W2SCALE, -15.0, 15.0).astype(FP8E3)

    # digitcaps weights V[h, s, p, ki]:
    #   oc2 = 128h+p; cap=oc2>>3; j=oc2&7; n = cap*16 + j*2 + (s>>3); jj = s&7
    Wd = W[0]  # (512, 10, 16, 8)
    oc2 = np.arange(256)
    n_base = (oc2 >> 3) * 16 + (oc2 & 7) * 2
    V = np.empty((2, 16, 128, 160), np.float32)
    for s in range(16):
        sel = Wd[n_base + (s >> 3), :, :, s & 7]      # (256, 10, 16)
        V[:, s] = sel.reshape(2, 128, 160)

    # pri_b rides the same W2SCALE so the shared PSUM descale recovers it
    pbw = (pri_b.reshape(1, 256) * W2SCALE).astype(BF16)  # (1, hh*128+p)

    # conv1 operands packed two-logical-rows-per-partition so the whole
    # thing moves as ONE full-rate 3KB-row DMA descriptor: partition p =
    # [row_p (384) | row_{128+p} (384, zero-padded past 243)].
    c1 = np.concatenate([im2col_e, W1T_e], axis=1).astype(BF16)  # (244, 768/2)
    c1p = np.zeros((128, 768), BF16)
    c1p[:, :384] = c1[:128]
    c1p[:116, 384:] = c1[128:244]
    shared = {
        "c1p": np.ascontiguousarray(c1p),
        "w2s": w2s,
        "pbw": np.ascontiguousarray(pbw),
    }
    per_core = []
    for c in range(NCORES):
        vsl = V[:, :, :, c * KI:(c + 1) * KI]                     # (2,16,128,20)
        vsl = vsl.transpose(2, 0, 1, 3).reshape(128, 32 * KI)     # (128, 640)
        d = dict(shared)
        d["v"] = np.ascontiguousarray(vsl).astype(BF16)
        per_core.append(d)
    return per_core


INPUT_SPECS = {
    "c1p": ((128, 768), BF),
    "w2s": ((128, 81 * 256), F8E3),
    "v": ((128, 32 * KI), BF),
    "pbw": ((1, 256), BF),
}


# --------------------------------------------------------------------------
# Device IR
# --------------------------------------------------------------------------

def emit_kernel(tc, out_ap, ins, taps=None):
    nc = tc.nc
    with (
        tc.tile_pool(name="sb", bufs=1) as sb,
        tc.tile_pool(name="ps", bufs=1, space="PSUM") as ps,
    ):
        # ---- DMA plan. The 16 SDMA engines time-slice all active rings at
        # packet-group granularity, so a side ring running alongside the w2
        # stream is starved to ~25GB/s: everything bulk must ride the sync
        # HWDGE ring in FIFO order, c1 FIRST (consumed first), then the w2
        # chunks. pbw (tiny) goes early on the scalar ring; v is emitted
        # late so its DMA-sem lane recycle waits on an early chunk, and its
        # starved transfer still lands long before digitcaps needs it.
        # Emission order pins the 8 round-robin DMA-sem lanes so that every
        # recycled lane points at an early-completing transfer.
        c1p_sb = sb.tile([128, 768], BF)
        pbw_sb = sb.tile([1, 256], BF)
        v_sb = sb.tile([128, 32 * KI], BF)
        nc.scalar.dma_start(pbw_sb[:], ins["pbw"][:])
        nc.sync.dma_start(c1p_sb[:], ins["c1p"][:])
        # The last two (tiny) chunks ride the starved scalar ring instead:
        # they trickle in mid-stream, so the PE's final gate is the sync
        # ring's last chunk, whose completion receipt lands ~1us earlier.
        w2t = []
        off = 0
        for j, cn in enumerate(W2CHUNKS):
            wt = sb.tile([128, cn * 256], F8E3, name=f"w2t{j}")
            ring = nc.scalar if j >= len(W2CHUNKS) - 2 else nc.sync
            ring.dma_start(wt[:], ins["w2s"][:, off * 256:(off + cn) * 256])
            w2t.append(wt)
            off += cn
        nc.scalar.dma_start(v_sb[:], ins["v"][:])
        # ones row for the pri_b bias matmuls
        ones16 = sb.tile([1, 16], BF)
        nc.gpsimd.memset(ones16[:], 1.0)
        # Dummy Sqrt: forces the sqrt act-table set to load NOW (scalar queue
        # is idle), not between the mid-squash Square and Sqrt where the
        # compiler would otherwise place it (a 1.3us mid-chain stall).
        dummy = sb.tile([1, 1], F32)
        nc.scalar.activation(dummy[:], ones16[0:1, 0:1],
                             mybir.ActivationFunctionType.Sqrt)

        # ---- conv1: h = W1T_e.T @ im2col_e (bias folded in)  -> (128,256)
        psum1 = ps.tile([128, 256], F32)
        nc.tensor.matmul(psum1[:], c1p_sb[:, 256:384], c1p_sb[:, 0:256],
                         start=True, stop=False)
        nc.tensor.matmul(psum1[:], c1p_sb[0:116, 640:768],
                         c1p_sb[0:116, 384:640],
                         start=False, stop=True)
        h_sb = sb.tile([128, 256], BF)
        nc.vector.tensor_copy(h_sb[:], psum1[:])
        h4 = h_sb[:].rearrange("p (y x) -> p y x", y=16)

        # ---- conv2: 81 strided-view matmuls per oc2-half, PSUM-accumulated.
        # The two halves are independent accumulation chains and a matmul's
        # start=True clears its WHOLE PSUM bank, so each half gets its own
        # bank inside one 2-bank tile (bank = 512 f32); a single strided
        # tensor_copy still pulls both out. pri_b folds in via two 1-row
        # matmuls mid-chain.
        psum2 = ps.tile([128, 1024], F32)
        chunk_of = []
        for j, cn in enumerate(W2CHUNKS):
            chunk_of += [(j, k) for k in range(cn)]
        for dydx in range(81):
            dy, dx = divmod(dydx, 9)
            j, jj = chunk_of[dydx]
            rhs = h4[:, dy:dy + 8:2, dx:dx + 8:2]
            for hh in range(2):
                nc.tensor.matmul(
                    psum2[:, hh * 512:hh * 512 + 16],
                    w2t[j][:, jj * 256 + hh * 128: jj * 256 + (hh + 1) * 128],
                    rhs,
                    start=(dydx == 0), stop=(dydx == 80),
                )
            if dydx == BIAS_AT:
                for hh in range(2):
                    nc.tensor.matmul(
                        psum2[:, hh * 512:hh * 512 + 16],
                        pbw_sb[:, hh * 128:(hh + 1) * 128], ones16[:],
                        start=False, stop=False,
                    )

        # ---- squash per (p, h, s_hi) group of 8:
        # f = sqrt(sq)/512 / (1+sq)   (1/512 cij folded in)
        x2b = sb.tile([128, 32], F32)
        nc.vector.tensor_scalar_mul(
            x2b[:].rearrange("p (b r) -> p b r", b=2),
            psum2[:].rearrange("p (b r) -> p b r", b=2)[:, :, 0:16],
            1.0 / W2SCALE,
        )
        # t2 = (psum2/W2SCALE)^2 on the Scalar engine (set-0 act table, loaded
        # early), overlapping the Vector descale copy above.
        t2 = sb.tile([128, 32], F32)
        nc.scalar.activation(
            t2[:].rearrange("p (b r) -> p b r", b=2),
            psum2[:].rearrange("p (b r) -> p b r", b=2)[:, :, 0:16],
            mybir.ActivationFunctionType.Square, scale=1.0 / W2SCALE,
        )
        sq = sb.tile([128, 4], F32)
        nc.vector.tensor_reduce(
            sq[:], t2[:].rearrange("p (g e) -> p g e", e=8),
            axis=mybir.AxisListType.X, op=mybir.AluOpType.add,
        )
        r_ = sb.tile([128, 4], F32)
        nc.scalar.activation(
            r_[:], sq[:], mybir.ActivationFunctionType.Sqrt,
            scale=1.0 / (512.0 * 512.0),
        )
        d2 = sb.tile([128, 4], F32)
        nc.vector.tensor_scalar_add(d2[:], sq[:], 1.0)
        rec2 = sb.tile([128, 4], F32)
        nc.vector.reciprocal(rec2[:], d2[:])
        f_ = sb.tile([128, 4], F32)
        nc.vector.tensor_mul(f_[:], r_[:], rec2[:])

        u_sb = sb.tile([128, 32], BF)
        nc.vector.tensor_mul(
            u_sb[:].rearrange("p (g e) -> p g e", e=8),
            x2b[:].rearrange("p (g e) -> p g e", e=8),
            f_[:].broadcast_to((128, 4, 8)),
        )

        # ---- digitcaps matvec: psum_d[0, ki] = sum_{h,s,p} u * V
        psum_d = ps.tile([1, KI], F32)
        for idx in range(32):
            nc.tensor.matmul(
                psum_d[:],
                u_sb[:, idx:idx + 1],
                v_sb[:, idx * KI:(idx + 1) * KI],
                start=(idx == 0), stop=(idx == 31),
            )

        # ---- final elementwise squash: vij = s*|s|/(1+s^2)
        # (s must be staged to SBUF first: a dual-PSUM-operand tensor_tensor
        # fails walrus codegen — PSUM has a single DVE read port.)
        s_sb = sb.tile([1, KI], F32)
        nc.vector.tensor_copy(s_sb[:], psum_d[:])
        t3 = sb.tile([1, KI], F32)
        nc.vector.tensor_mul(t3[:], s_sb[:], s_sb[:])
        d3 = sb.tile([1, KI], F32)
        nc.vector.tensor_scalar_add(d3[:], t3[:], 1.0)
        rec3 = sb.tile([1, KI], F32)
        nc.vector.reciprocal(rec3[:], d3[:])
        a3 = sb.tile([1, KI], F32)
        nc.scalar.activation(a3[:], t3[:], mybir.ActivationFunctionType.Sqrt)
        m3 = sb.tile([1, KI], F32)
        nc.vector.tensor_mul(m3[:], a3[:], s_sb[:])
        o3 = sb.tile([1, KI], F32)
        nc.vector.tensor_mul(o3[:], m3[:], rec3[:])
        nc.sync.dma_start(out_ap[:], o3[:])

        if taps is not None:
            for name, t in (("x2b", x2b), ("u", u_sb), ("s", s_sb),
                            ("h", h_sb)):
                nc.scalar.dma_start(taps[name][:], t[:])


# --------------------------------------------------------------------------
# Build + run
# --------------------------------------------------------------------------

_CACHE = {}


def build_nc():
    nc = bacc.Bacc(
        "TRN2", target_bir_lowering=False, debug=False, num_devices=NCORES
    )
    ins = {
        name: nc.dram_tensor(name, list(shape), dt, kind="ExternalInput").ap()
        for name, (shape, dt) in INPUT_SPECS.items()
    }
    out_ap = nc.dram_tensor("out", [1, KI], F32, kind="ExternalOutput").ap()
    taps = None
    if DEBUG_TAPS:
        taps = {
            "x2b": nc.dram_tensor("tap_x2b", [128, 32], F32,
                                  kind="ExternalOutput").ap(),
            "u": nc.dram_tensor("tap_u", [128, 32], BF,
                                kind="ExternalOutput").ap(),
            "s": nc.dram_tensor("tap_s", [1, KI], F32,
                                kind="ExternalOutput").ap(),
            "h": nc.dram_tensor("tap_h", [128, 256], BF,
                                kind="ExternalOutput").ap(),
        }
    with FastTailTileContext(nc) as tc:
        emit_kernel(tc, out_ap, ins, taps=taps)
    nc.compile()
    return nc


def kernel(**inputs):
    per_core = _host_prep(**inputs)
    if "nc" not in _CACHE:
        _CACHE["nc"] = build_nc()
    res = run_bass_kernel_spmd(
        _CACHE["nc"], per_core, core_ids=list(range(NCORES))
    )
    out = np.concatenate(
        [np.asarray(res.results[c]["out"], np.float32).reshape(-1)
         for c in range(NCORES)]
    )
    return out.reshape(1, 1, 10, 16, 1)


# revision 20
# speedup vs baseline: 1.0454x; 1.0454x over previous
"""Trainium2 Bass kernel for nn_CapsNet_69114613730132.

Strategy (8 NeuronCores, SPMD, zero collectives):
  The CapsNet routing loop is degenerate (self.bij is never updated, so
  cij stays 1/512) and collapses to: conv1 -> conv2 -> squash ->
  4096->160 matvec -> elementwise squash. The convolutions are tiny, so
  cross-core collectives (AllGather floor + a ~40us rank-alignment
  barrier measured on this fabric) cost more than replicating them.

  * Every core computes conv1 + conv2 (PrimaryCaps) + squash redundantly:
      conv1 as a 244-contraction matmul over a host-built im2col of x
      (row 243 = ones X conv_b, folding the bias into the matmul);
      conv2 as 81 (dy,dx) PSUM-accumulated matmuls over strided views of
      h (no im2col materialization), weights stationary, bf16; pri_b
      folded in via two 1-row matmuls against a ones vector.
  * The DigitCaps matvec output (160 = 10*16) is sharded 20-per-core via
    per-core weight slices => cores are fully independent; the host just
    concatenates the 8 (1,20) results. No communication at all.
  * All PE compute in bf16 (weights host-cast), f32 PSUM/vector math.

Perf notes (vs the 36us single-ring baseline):
  * The kernel is bound by the 5.3MB conv2 weight stream from HBM
    (~13us at the ~420GB/s per-core aggregate DMA peak). The 16 DMA
    engines time-slice all active rings, so extra rings don't add
    bandwidth -- but the small-packet c1/v transfers on the same ring
    as w2 stalled the stream and (worse) the scalar ring was so slow
    that c1_b/v landed after 24us. Now: all w2 chunks stream on the
    sync ring; c1/pbw/v go on the gpsimd ring early; the scalar ring
    carries nothing so its act-table loads stay off the critical path.
  * Only Sqrt activations are used (one table set): a second table set
    would be reloaded mid-squash-chain (~1.3us stall on first use).
  * Chunks shrink toward the tail (last four are 1 dydx = 65KB) so the
    PE's in-order accumulation trails the final byte by ~0.2us.

kernel(**inputs) takes the FULL unsharded inputs and returns the full
(1,1,10,16,1) float32 output.
"""
import numpy as np
import ml_dtypes

import concourse.bass as bass
import concourse.bacc as bacc
import concourse.tile as tile
import concourse.mybir as mybir
from concourse.bass_utils import run_bass_kernel_spmd
from concourse.tile import ScopedClock, add_dep_helper

FAST_TAIL = True
DEBUG_TAPS = False   # set True (before build_nc) to export x2b/u/s taps


class FastTailTileContext(tile.TileContext):
    """TileContext tail with a 1-hop handshake instead of the all-engine
    barriers (each an EVSEM polling butterfly measured at ~7us here).

    The sync.drain waits for every tracked semaphore target, so by the
    time it passes, every sem-touching instruction on every engine has
    retired (each engine's last real work is upstream of the output DMA
    the drain waits on). A single drain->GpSimd semaphore hop then orders
    the sem/DMA-state clears; the next execution's NEFF entry barrier
    orders everything else."""

    def _drain_and_barrier(self, tick_clock, wait_clock):
        if not FAST_TAIL:
            return super()._drain_and_barrier(tick_clock, wait_clock)
        nc = self.nc
        # GpSimd (the clearing engine, otherwise idle here) waits on every
        # tracked semaphore's final value itself, then clears.
        drain_inst = nc.gpsimd.drain()
        wait_clock.add_sem_waits(
            drain_inst.ins, ScopedClock({None: tick_clock.global_clock})
        )
        # DMA completion sems update asynchronously and the final DMA (the
        # output store) has no downstream consumer, so the clock misses it:
        # wait each DMA-lane sem out to its summed final value explicitly.
        dma_totals = {}
        for insts in self.ordered_instructions_by_block.values():
            for i in insts:
                si = i.sync_info
                if si is None or not si.on_update:
                    continue
                for u in si.on_update:
                    if (u.sync_type == "semaphore" and u.update_value
                            and (u.ant_name or "").startswith("DMA")):
                        k = (u.id, u.ant_name)
                        dma_totals[k] = dma_totals.get(k, 0) + u.update_value
        handles = {h.num: h for h in self.sems.allocated().values()}
        for eng in (nc.gpsimd, nc.sync, nc.tensor, nc.vector, nc.scalar):
            for (sid, _), tot in sorted(dma_totals.items()):
                if sid in handles:
                    eng.wait_ge(handles[sid], tot)
        popped = nc._tile_sem_poison_stack.pop()
        assert popped is self._sem_poison
        nc.clear_and_free_semaphores(list(self.sems.allocated().values()))

BF16 = ml_dtypes.bfloat16
FP8E3 = ml_dtypes.float8_e3m4
F32 = mybir.dt.float32
BF = mybir.dt.bfloat16
F8E3 = mybir.dt.float8e3

NCORES = 8
KI = 20             # digitcaps output elems per core (160 = 8*20)
# dydx positions per w2 DMA chunk, all on the sync ring. Descending
# sizes: big chunks amortize descriptor issue (~650ns each), the 1-dydx
# tail chunks keep the PE right behind the stream end.
W2CHUNKS = [24, 24, 16, 8, 4, 3, 2]
assert sum(W2CHUNKS) == 81
# conv2 weights stream as fp8 E3M4 (4 mantissa bits), scaled by W2SCALE on
# the host into E3M4's normal range (values ~N(0, 0.02)*128 => ~N(0, 2.56),
# |max| ~ 13 < 15.5) and descaled at the PSUM->SBUF copy. The PE upcasts
# each operand independently, so h stays bf16. Halves the HBM stream.
W2SCALE = 128.0
# dydx index after which the two pri_b bias matmuls are inserted into the
# conv2 PSUM accumulation (late enough that pbw's gpsimd-ring DMA has
# landed, well before the stream ends).
BIAS_AT = 48


# --------------------------------------------------------------------------
# Host-side input marshalling (pure layout transforms + dtype casts)
# --------------------------------------------------------------------------

def _host_prep(x, conv_w, conv_b, pri_w, pri_b, W):
    x = np.asarray(x, np.float32)
    conv_w = np.asarray(conv_w, np.float32)
    conv_b = np.asarray(conv_b, np.float32)
    pri_w = np.asarray(pri_w, np.float32)
    pri_b = np.asarray(pri_b, np.float32)
    W = np.asarray(W, np.float32)

    # im2col of x: (243, 256), row (c,dy,dx), col (oy*16+ox)
    im2col1 = np.empty((3, 9, 9, 16, 16), np.float32)
    for dy in range(9):
        for dx in range(9):
            im2col1[:, dy, dx] = x[0, :, dy:dy + 16, dx:dx + 16]
    im2col1 = im2col1.reshape(243, 256)

    W1T = conv_w.reshape(128, 243).T  # (243, 128)

    # 244th contraction row folds conv_b into the conv1 matmul:
    # im2col row = ones, weight row = conv_b.
    im2col_e = np.concatenate([im2col1, np.ones((1, 256), np.float32)])
    W1T_e = np.concatenate([W1T, conv_b.reshape(1, 128)])

    # (ic, dydx*256 + oc2) with oc2 = cap*8 + j
    w2s = (pri_w.reshape(256, 128, 9, 9)
           .transpose(2, 3, 1, 0)          # (dy, dx, ic, oc2)
           .reshape(81, 128, 256)
           .transpose(1, 0, 2)             # (ic, dydx, oc2)
           .reshape(128, 81 * 256))
    w2s = np.clip(w2s * W2SCALE, -15.0, 15.0).astype(FP8E3)

    # digitcaps weights V[h, s, p, ki]:
    #   oc2 = 128h+p; cap=oc2>>3; j=oc2&7; n = cap*16 + j*2 + (s>>3); jj = s&7
    Wd = W[0]  # (512, 10, 16, 8)
    oc2 = np.arange(256)
    n_base = (oc2 >> 3) * 16 + (oc2 & 7) * 2
    V = np.empty((2, 16, 128, 160), np.float32)
    for s in range(16):
        sel = Wd[n_base + (s >> 3), :, :, s & 7]      # (256, 10, 16)
        V[:, s] = sel.reshape(2, 128, 160)

    # pri_b rides the same W2SCALE so the shared PSUM descale recovers it
    pbw = (pri_b.reshape(1, 256) * W2SCALE).astype(BF16)  # (1, hh*128+p)

    # conv1 operands packed two-logical-rows-per-partition so the whole
    # thing moves as ONE full-rate 3KB-row DMA descriptor: partition p =
    # [row_p (384) | row_{128+p} (384, zero-padded past 243)].
    c1 = np.concatenate([im2col_e, W1T_e], axis=1).astype(BF16)  # (244, 768/2)
    c1p = np.zeros((128, 768), BF16)
    c1p[:, :384] = c1[:128]
    c1p[:116, 384:] = c1[128:244]
    shared = {
        "c1p": np.ascontiguousarray(c1p),
        "w2s": w2s,
        "pbw": np.ascontiguousarray(pbw),
    }
    per_core = []
    for c in range(NCORES):
        vsl = V[:, :, :, c * KI:(c + 1) * KI]                     # (2,16,128,20)
        vsl = vsl.transpose(2, 0, 1, 3).reshape(128, 32 * KI)     # (128, 640)
        d = dict(shared)
        d["v"] = np.ascontiguousarray(vsl).astype(BF16)
        per_core.append(d)
    return per_core


INPUT_SPECS = {
    "c1p": ((128, 768), BF),
    "w2s": ((128, 81 * 256), F8E3),
    "v": ((128, 32 * KI), BF),
    "pbw": ((1, 256), BF),
}


# --------------------------------------------------------------------------
# Device IR
# --------------------------------------------------------------------------

def emit_kernel(tc, out_ap, ins, taps=None):
    nc = tc.nc
    with (
        tc.tile_pool(name="sb", bufs=1) as sb,
        tc.tile_pool(name="ps", bufs=1, space="PSUM") as ps,
    ):
        # ---- DMA plan. The 16 SDMA engines time-slice all active rings at
        # packet-group granularity, so a side ring running alongside the w2
        # stream is starved to ~25GB/s: everything bulk must ride the sync
        # HWDGE ring in FIFO order, c1 FIRST (consumed first), then the w2
        # chunks. pbw (tiny) goes early on the scalar ring; v is emitted
        # late so its DMA-sem lane recycle waits on an early chunk, and its
        # starved transfer still lands long before digitcaps needs it.
        # Emission order pins the 8 round-robin DMA-sem lanes so that every
        # recycled lane points at an early-completing transfer.
        c1p_sb = sb.tile([128, 768], BF)
        pbw_sb = sb.tile([1, 256], BF)
        v_sb = sb.tile([128, 32 * KI], BF)
        nc.scalar.dma_start(pbw_sb[:], ins["pbw"][:])
        nc.sync.dma_start(c1p_sb[:], ins["c1p"][:])
        w2t = []
        off = 0
        for j, cn in enumerate(W2CHUNKS):
            wt = sb.tile([128, cn * 256], F8E3, name=f"w2t{j}")
            nc.sync.dma_start(
                wt[:], ins["w2s"][:, off * 256:(off + cn) * 256])
            w2t.append(wt)
            off += cn
        nc.scalar.dma_start(v_sb[:], ins["v"][:])
        # ones row for the pri_b bias matmuls
        ones16 = sb.tile([1, 16], BF)
        nc.gpsimd.memset(ones16[:], 1.0)
        # Dummy Sqrt: forces the sqrt act-table set to load NOW (scalar queue
        # is idle), not between the mid-squash Square and Sqrt where the
        # compiler would otherwise place it (a 1.3us mid-chain stall).
        dummy = sb.tile([1, 1], F32)
        nc.scalar.activation(dummy[:], ones16[0:1, 0:1],
                             mybir.ActivationFunctionType.Sqrt)

        # ---- conv1: h = W1T_e.T @ im2col_e (bias folded in)  -> (128,256)
        psum1 = ps.tile([128, 256], F32)
        nc.tensor.matmul(psum1[:], c1p_sb[:, 256:384], c1p_sb[:, 0:256],
                         start=True, stop=False)
        nc.tensor.matmul(psum1[:], c1p_sb[0:116, 640:768],
                         c1p_sb[0:116, 384:640],
                         start=False, stop=True)
        h_sb = sb.tile([128, 256], BF)
        nc.vector.tensor_copy(h_sb[:], psum1[:])
        h4 = h_sb[:].rearrange("p (y x) -> p y x", y=16)

        # ---- conv2: 81 strided-view matmuls per oc2-half, PSUM-accumulated.
        # The two halves are independent accumulation chains and a matmul's
        # start=True clears its WHOLE PSUM bank, so each half gets its own
        # bank inside one 2-bank tile (bank = 512 f32); a single strided
        # tensor_copy still pulls both out. pri_b folds in via two 1-row
        # matmuls mid-chain.
        psum2 = ps.tile([128, 1024], F32)
        chunk_of = []
        for j, cn in enumerate(W2CHUNKS):
            chunk_of += [(j, k) for k in range(cn)]
        for dydx in range(81):
            dy, dx = divmod(dydx, 9)
            j, jj = chunk_of[dydx]
            rhs = h4[:, dy:dy + 8:2, dx:dx + 8:2]
            for hh in range(2):
                nc.tensor.matmul(
                    psum2[:, hh * 512:hh * 512 + 16],
                    w2t[j][:, jj * 256 + hh * 128: jj * 256 + (hh + 1) * 128],
                    rhs,
                    start=(dydx == 0), stop=(dydx == 80),
                )
            if dydx == BIAS_AT:
                for hh in range(2):
                    nc.tensor.matmul(
                        psum2[:, hh * 512:hh * 512 + 16],
                        pbw_sb[:, hh * 128:(hh + 1) * 128], ones16[:],
                        start=False, stop=False,
                    )

        # ---- squash per (p, h, s_hi) group of 8:
        # f = sqrt(sq)/512 / (1+sq)   (1/512 cij folded in)
        x2b = sb.tile([128, 32], F32)
        nc.vector.tensor_scalar_mul(
            x2b[:].rearrange("p (b r) -> p b r", b=2),
            psum2[:].rearrange("p (b r) -> p b r", b=2)[:, :, 0:16],
            1.0 / W2SCALE,
        )
        # t2 = (psum2/W2SCALE)^2 on the Scalar engine (set-0 act table, loaded
        # early), overlapping the Vector descale copy above.
        t2 = sb.tile([128, 32], F32)
        nc.scalar.activation(
            t2[:].rearrange("p (b r) -> p b r", b=2),
            psum2[:].rearrange("p (b r) -> p b r", b=2)[:, :, 0:16],
            mybir.ActivationFunctionType.Square, scale=1.0 / W2SCALE,
        )
        sq = sb.tile([128, 4], F32)
        nc.vector.tensor_reduce(
            sq[:], t2[:].rearrange("p (g e) -> p g e", e=8),
            axis=mybir.AxisListType.X, op=mybir.AluOpType.add,
        )
        r_ = sb.tile([128, 4], F32)
        nc.scalar.activation(
            r_[:], sq[:], mybir.ActivationFunctionType.Sqrt,
            scale=1.0 / (512.0 * 512.0),
        )
        d2 = sb.tile([128, 4], F32)
        nc.vector.tensor_scalar_add(d2[:], sq[:], 1.0)
        rec2 = sb.tile([128, 4], F32)
        nc.vector.reciprocal(rec2[:], d2[:])
        f_ = sb.tile([128, 4], F32)
        nc.vector.tensor_mul(f_[:], r_[:], rec2[:])

        u_sb = sb.tile([128, 32], BF)
        nc.vector.tensor_mul(
            u_sb[:].rearrange("p (g e) -> p g e", e=8),
            x2b[:].rearrange("p (g e) -> p g e", e=8),
            f_[:].broadcast_to((128, 4, 8)),
        )

        # ---- digitcaps matvec: psum_d[0, ki] = sum_{h,s,p} u * V
        psum_d = ps.tile([1, KI], F32)
        for idx in range(32):
            nc.tensor.matmul(
                psum_d[:],
                u_sb[:, idx:idx + 1],
                v_sb[:, idx * KI:(idx + 1) * KI],
                start=(idx == 0), stop=(idx == 31),
            )

        # ---- final elementwise squash: vij = s*|s|/(1+s^2)
        # (s must be staged to SBUF first: a dual-PSUM-operand tensor_tensor
        # fails walrus codegen — PSUM has a single DVE read port.)
        s_sb = sb.tile([1, KI], F32)
        nc.vector.tensor_copy(s_sb[:], psum_d[:])
        t3 = sb.tile([1, KI], F32)
        nc.vector.tensor_mul(t3[:], s_sb[:], s_sb[:])
        d3 = sb.tile([1, KI], F32)
        nc.vector.tensor_scalar_add(d3[:], t3[:], 1.0)
        rec3 = sb.tile([1, KI], F32)
        nc.vector.reciprocal(rec3[:], d3[:])
        a3 = sb.tile([1, KI], F32)
        nc.scalar.activation(a3[:], t3[:], mybir.ActivationFunctionType.Sqrt)
        m3 = sb.tile([1, KI], F32)
        nc.vector.tensor_mul(m3[:], a3[:], s_sb[:])
        o3 = sb.tile([1, KI], F32)
        nc.vector.tensor_mul(o3[:], m3[:], rec3[:])
        nc.sync.dma_start(out_ap[:], o3[:])

        if taps is not None:
            for name, t in (("x2b", x2b), ("u", u_sb), ("s", s_sb),
                            ("h", h_sb)):
                nc.scalar.dma_start(taps[name][:], t[:])


# --------------------------------------------------------------------------
# Build + run
# --------------------------------------------------------------------------

_CACHE = {}


def build_nc():
    nc = bacc.Bacc(
        "TRN2", target_bir_lowering=False, debug=False, num_devices=NCORES
    )
    ins = {
        name: nc.dram_tensor(name, list(shape), dt, kind="ExternalInput").ap()
        for name, (shape, dt) in INPUT_SPECS.items()
    }
    out_ap = nc.dram_tensor("out", [1, KI], F32, kind="ExternalOutput").ap()
    taps = None
    if DEBUG_TAPS:
        taps = {
            "x2b": nc.dram_tensor("tap_x2b", [128, 32], F32,
                                  kind="ExternalOutput").ap(),
            "u": nc.dram_tensor("tap_u", [128, 32], BF,
                                kind="ExternalOutput").ap(),
            "s": nc.dram_tensor("tap_s", [1, KI], F32,
                                kind="ExternalOutput").ap(),
            "h": nc.dram_tensor("tap_h", [128, 256], BF,
                                kind="ExternalOutput").ap(),
        }
    with FastTailTileContext(nc) as tc:
        emit_kernel(tc, out_ap, ins, taps=taps)
    nc.compile()
    return nc


def kernel(**inputs):
    per_core = _host_prep(**inputs)
    if "nc" not in _CACHE:
        _CACHE["nc"] = build_nc()
    res = run_bass_kernel_spmd(
        _CACHE["nc"], per_core, core_ids=list(range(NCORES))
    )
    out = np.concatenate(
        [np.asarray(res.results[c]["out"], np.float32).reshape(-1)
         for c in range(NCORES)]
    )
    return out.reshape(1, 1, 10, 16, 1)
